# revision 88
# baseline (speedup 1.0000x reference)
"""Trainium2 Bass kernel for nn_Cvxnnregression.

Reference computation (per sample, batch B=131072):
  r = MLP(x):  16 -> 100 -> 100 -> 100 -> 4   (relu between, bias everywhere)
  head: two independent 2x2 diagonal-preconditioned solves built from
        closed-form scalar arithmetic on x components and r.
  outputs: (relu(p), r)  both [B, 4] fp32.

Strategy: pure data parallel over 8 NeuronCores (16384 samples each).
Feature-major MLP via PE matmuls (batch on the free dim). The p output is
several-hundred-fold more sensitive to r errors than the 2e-2 gate
suggests, so reduced-precision matmul dtypes (f32r/bf16/fp16) fail
outright; instead every fp32 matmul is emulated with fp16 split pairs at
full-rate (1 cycle/row) PE throughput:

  value = hi + lo with hi = fp16(value), lo = fp16(value - hi)  (~22-24
  mantissa bits. PE fp16 multiplies accumulate exactly into fp32 PSUM and
  handle the subnormal residuals correctly - measured ~1.8x fp32 error.)

  - L1: hi/lo of x interleaved per sample in `xpair`, so one PE transpose
    per 128 samples yields a K=32 stacked moving tile; TWO stacked
    matmuls [W1hi;W1lo] and [W1lo;W1hi] compute all four split products:
    2 cycles/row vs 4 for fp32.
  - L2/L3: h kept fp32 only transiently; its fp16 pair (hhi, hlo) is made
    by one SBUF copy + one subtract, then 3 matmuls Whi*hhi + Whi*hlo +
    Wlo*hhi accumulate in one PSUM bank (the dropped Wlo*hlo term is
    ~2^-24 relative): 3 cycles/row vs 4.
  - r-layer: sample-major emission via swapped operands (stationary = h3
    block, moving = Wr) in plain fp32: 16 PE cycles per 128 samples.

Column order convention: h-matrix column c = q*128 + p corresponds to
sample s = p*128 + q (a perfect shuffle), so transpose tiles and r/p
output tiles line up with contiguous DMA layouts on both ends.

The three layers run as a skewed software pipeline, L1(t) | L2(t-3) |
L3(t-8), so the fp16 pairs are consumed a few steps after production
(small rotating pools suffice) and all engines stay loaded end-to-end.
Elementwise work is placed by memory-port legality: PSUM-reading ops
(relu+bias copies, xs copies, r bias) go on ACT/DVE; SBUF-only ops
(pair construction, head arithmetic) go mostly on Pool/gpsimd, which
has no PSUM port on TRN2. The fp16 weight splits (including the
K-stacked W1 operands) are precomputed on the host in kernel() and
loaded by plain DMAs, so no on-device assembly gates the pipeline.
"""

import os
import sys

for _p in ("/opt/trn_rl_repo", "/root/.axon_site/_ro/trn_rl_repo"):
    if os.path.isdir(_p) and _p not in sys.path:
        sys.path.insert(0, _p)

import numpy as np

import concourse.bass as bass
import concourse.mybir as mybir
import concourse.tile as tile
from concourse.masks import make_identity

N_CORES = 8
B_LOC = 16384          # samples per core
D_IN = 16
NH = 100               # hidden width
NB = 32                # number of 512-wide column tiles per core
F32 = mybir.dt.float32
FP16 = mybir.dt.float16
ALU = mybir.AluOpType
AFT = mybir.ActivationFunctionType


def _legalize_waits(nc, max_waits=1):
    """Split multi-sem-wait instructions for this container's walrus build.

    The walrus here rejects more than one sync wait per instruction ("Too
    many sync wait commands"), while Tile freely packs several. Hoist the
    extra waits onto same-engine NoOps inserted immediately before the
    instruction — the engine stalls on the NoOps first, so ordering
    semantics are identical.
    """
    n = 0
    for f in nc.m.functions:
        for bb in f.blocks:
            out = []
            changed = False
            for inst in bb.instructions:
                si = inst.sync_info
                if si is not None and len(si.on_wait) > max_waits:
                    waits = list(si.on_wait)
                    for w in waits[max_waits:]:
                        n += 1
                        nop = mybir.InstNoOp(name=f"Zw-{n}", ins=[], outs=[])
                        nop.engine = inst.engine
                        nop.sync_info = mybir.SyncInfo(on_wait=[w], on_update=[])
                        out.append(nop)
                    inst.sync_info = mybir.SyncInfo(
                        on_wait=waits[:max_waits], on_update=list(si.on_update)
                    )
                    changed = True
                out.append(inst)
            if changed:
                bb.instructions = out
    return n


def _build(legalize=True):
    nc = bass.Bass(trn_type="TRN2")

    xd = nc.dram_tensor("x", [B_LOC, D_IN], F32, kind="ExternalInput")
    w1d = nc.dram_tensor("W1", [D_IN, NH], F32, kind="ExternalInput")
    b1d = nc.dram_tensor("b1", [NH], F32, kind="ExternalInput")
    w2d = nc.dram_tensor("W2", [NH, NH], F32, kind="ExternalInput")
    b2d = nc.dram_tensor("b2", [NH], F32, kind="ExternalInput")
    w3d = nc.dram_tensor("W3", [NH, NH], F32, kind="ExternalInput")
    b3d = nc.dram_tensor("b3", [NH], F32, kind="ExternalInput")
    wrd = nc.dram_tensor("Wr", [NH, 4], F32, kind="ExternalInput")
    brd = nc.dram_tensor("br", [4], F32, kind="ExternalInput")
    # host-precomputed fp16 split operands (see kernel()): K-stacked L1
    # pairs and the L2/L3 hi/lo weight splits, loaded by plain DMAs.
    w1ad = nc.dram_tensor("W1s_a", [2 * D_IN, NH], FP16, kind="ExternalInput")
    w1bd = nc.dram_tensor("W1s_b", [2 * D_IN, NH], FP16, kind="ExternalInput")
    w2hid = nc.dram_tensor("W2hi", [NH, NH], FP16, kind="ExternalInput")
    w2lod = nc.dram_tensor("W2lo", [NH, NH], FP16, kind="ExternalInput")
    w3hid = nc.dram_tensor("W3hi", [NH, NH], FP16, kind="ExternalInput")
    w3lod = nc.dram_tensor("W3lo", [NH, NH], FP16, kind="ExternalInput")
    pod = nc.dram_tensor("p_out", [B_LOC, 4], F32, kind="ExternalOutput")
    rod = nc.dram_tensor("r_out", [B_LOC, 4], F32, kind="ExternalOutput")

    xv = xd.rearrange("(p n) d -> p (n d)", p=128)     # [128, 2048]
    pov = pod.rearrange("(p n) d -> p (n d)", p=128)   # [128, 512]
    rov = rod.rearrange("(p n) d -> p (n d)", p=128)   # [128, 512]

    with tile.TileContext(nc) as tc:
        with (
            tc.tile_pool(name="wpool", bufs=1) as wpool,
            tc.tile_pool(name="xpool", bufs=1) as xpool,
            tc.tile_pool(name="h3pool", bufs=1) as h3pool,
            tc.tile_pool(name="hfpool", bufs=10) as hfpool,
            tc.tile_pool(name="hpair", bufs=10) as hpair,
            tc.tile_pool(name="xspool", bufs=6) as xspool,
            tc.tile_pool(name="opool", bufs=1) as opool,
            tc.tile_pool(name="scr", bufs=2) as scr,
            tc.tile_pool(name="txps", bufs=3, space="PSUM") as txps,
            tc.tile_pool(name="mmps", bufs=4, space="PSUM") as mmps,
            tc.tile_pool(name="rps", bufs=1, space="PSUM") as rps,
        ):
            # ---- input x first: its DMAs gate the whole PE pipeline.
            # Contiguous destination (no pad) keeps the DMA at full rate.
            x_sb = xpool.tile([128, 2048], F32)
            xs3 = x_sb[:].rearrange("p (n d) -> p n d", d=16)
            xsrc = xv.rearrange("p (n d) -> p n d", d=16)
            XCH = [(0, 8), (8, 32), (32, 64), (64, 96), (96, 128)]
            for c0, c1 in XCH:
                nc.sync.dma_start(xs3[:, c0:c1, :], xsrc[:, c0:c1, :])

            # ---- weight splits precomputed on host: plain DMAs ----
            w1a = wpool.tile([2 * D_IN, NH], FP16)
            nc.scalar.dma_start(w1a[:], w1ad[:, :])
            w1b = wpool.tile([2 * D_IN, NH], FP16)
            nc.scalar.dma_start(w1b[:], w1bd[:, :])
            b1s = wpool.tile([NH, 1], F32)
            nc.scalar.dma_start(b1s[:], b1d.rearrange("(p o) -> p o", o=1))

            # ---- remaining weights on SP after x (needed only by ~15us)
            wrs = wpool.tile([NH, 4], F32)
            nc.sync.dma_start(wrs[:], wrd[:, :])
            b2s = wpool.tile([NH, 1], F32)
            nc.sync.dma_start(b2s[:], b2d.rearrange("(p o) -> p o", o=1))
            b3s = wpool.tile([NH, 1], F32)
            nc.sync.dma_start(b3s[:], b3d.rearrange("(p o) -> p o", o=1))
            br1 = wpool.tile([1, 4], F32)
            nc.sync.dma_start(br1[:], brd.rearrange("(o j) -> o j", o=1))

            w2hi = wpool.tile([NH, NH], FP16)
            nc.sync.dma_start(w2hi[:], w2hid[:, :])
            w2lo = wpool.tile([NH, NH], FP16)
            nc.sync.dma_start(w2lo[:], w2lod[:, :])
            w3hi = wpool.tile([NH, NH], FP16)
            nc.sync.dma_start(w3hi[:], w3hid[:, :])
            w3lo = wpool.tile([NH, NH], FP16)
            nc.sync.dma_start(w3lo[:], w3lod[:, :])

            ident = wpool.tile([128, 128], F32)
            make_identity(nc, ident[:])
            identh = wpool.tile([128, 128], FP16)
            nc.vector.tensor_copy(identh[:], ident[:])
            ones1 = wpool.tile([1, 128], F32)
            nc.gpsimd.memset(ones1[:], 1.0)

            # partition-broadcast br: [128, 4] with br[j] in every partition
            bc_ps = rps.tile([128, 4], F32, tag="rp")
            nc.tensor.matmul(bc_ps[:], ones1[:], br1[:])
            brbc = wpool.tile([128, 4], F32)
            nc.vector.tensor_copy(brbc[:], bc_ps[:])
            br64 = wpool.tile([128, 64], F32)
            nc.vector.tensor_copy(
                br64[:],
                bass.AP(brbc[:].tensor, brbc[:].offset,
                        [brbc[:].ap[0], [0, 16], [1, 4]]),
            )

            # ---- xpair fp16 [128, 4096]: block n = [hi16 | lo16] ----
            xpair = xpool.tile([128, 4096], FP16)
            xp3 = xpair[:].rearrange("p (n d) -> p n d", d=32)
            for k, (c0, c1) in enumerate(XCH):
                hi = xp3[:, c0:c1, 0:16]
                lo = xp3[:, c0:c1, 16:32]
                src = xs3[:, c0:c1, :]
                if k % 2 == 0:
                    nc.gpsimd.tensor_copy(hi, src)
                else:
                    nc.vector.tensor_copy(hi, src)
                nc.gpsimd.tensor_tensor(lo, src, hi, ALU.subtract)

            # ---- h3: persistent fp32 chunks (r-layer stationary) ----
            h3 = [h3pool.tile([NH, 2048], F32, tag=f"h3_{i}", name=f"h3_{i}")
                  for i in range(8)]

            r_out = opool.tile([128, 512], F32)
            p_out = opool.tile([128, 512], F32)
            pair = {0: {}, 1: {}}

            relu_ctr = [0]

            def relu_copy(dst_ap, src_ap, bias_ap):
                # bias + relu, PSUM -> SBUF; ACT is cheaper per op, takes 5/8
                i = relu_ctr[0]
                relu_ctr[0] += 1
                if i % 8 in (0, 2, 3, 5, 6):
                    nc.scalar.activation(dst_ap, src_ap, AFT.Relu, bias=bias_ap)
                else:
                    nc.vector.tensor_scalar(
                        dst_ap, src_ap, bias_ap, 0.0, ALU.add, ALU.max
                    )

            def make_pair(li, t, h_ap):
                """fp16 (hi, lo) pair of a [NH, 512] fp32 tile."""
                hhi = hpair.tile([NH, 512], FP16, tag=f"hi{li}",
                                 name=f"h{li}hi{t}")
                # SBUF-only: 1/4 DVE (2x_2p mode), 3/4 Pool
                if t % 4 == 0:
                    nc.vector.tensor_copy(hhi[:], h_ap)
                else:
                    nc.gpsimd.tensor_copy(hhi[:], h_ap)
                hlo = hpair.tile([NH, 512], FP16, tag=f"lo{li}",
                                 name=f"h{li}lo{t}")
                nc.gpsimd.tensor_tensor(hlo[:], h_ap, hhi[:], ALU.subtract)
                pair[li][t] = (hhi, hlo)

            # ---- layer-1 step: PE transposes + stacked matmuls ----
            # tile t: 4 PE transposes (fp16, 1 cycle/row) build the K=32
            # stacked moving tile [hi;lo] x 512 samples, then 2 stacked
            # matmuls compute the full split product.
            xs_d = {}

            def l1_fetch(t):
                # double-block transposes: charged by OUTPUT free size (128
                # rows) regardless of input width, so [128,64]->[64,128]
                # moves two sample blocks per instruction. The odd blocks
                # land at partition base 32; an SP-queue DMA relocates them
                # while an engine copy handles the even blocks - prefetched
                # ahead of the matmuls so the DMA latency is hidden.
                tx = txps.tile([64, 256], FP16, tag="tx", name=f"tx{t}")
                for h in range(2):
                    nc.tensor.transpose(
                        tx[:, 128 * h:128 * (h + 1)],
                        xpair[:, 128 * t + 64 * h:128 * t + 64 * (h + 1)],
                        identh[:],
                    )
                xs = xspool.tile([32, 512], FP16, tag="xs", name=f"xs{t}")
                xsv = xs[:].rearrange("p (a b) -> p a b", b=128)
                nc.scalar.copy(xsv[:, 0::2, :], tx[0:32, :])
                nc.vector.tensor_copy(xsv[:, 1::2, :], tx[32:64, :])
                xs_d[t] = xs

            def l1_step(t):
                xs = xs_d.pop(t)
                l1ps = mmps.tile([NH, 512], F32, tag="mm", name=f"l1ps{t}")
                nc.tensor.matmul(l1ps[:], w1a[:], xs[:],
                                 start=True, stop=False)
                nc.tensor.matmul(l1ps[:], w1b[:], xs[:],
                                 start=False, stop=True)
                h1f = hfpool.tile([NH, 512], F32, tag="h1f", name=f"h1f{t}")
                relu_copy(h1f[:], l1ps[:], b1s[:, 0:1])
                make_pair(0, t, h1f[:])

            # ---- layer r emission (sample-major via swapped operands) ----
            # one PSUM tile per PAIR of 32-col groups: halves the allocation
            # rate on the single-bank rp pool so its WAR wait trails by ~4
            # pipeline steps instead of gating the PE.
            def lr_pair(ch):
                r_ps = rps.tile([128, 64], F32, tag="rp", name=f"rps{ch}")
                for w in range(16):
                    u = 16 * ch + w
                    nc.tensor.matmul(
                        r_ps[:, 4 * w:4 * w + 4],
                        h3[u // 16][:, 128 * (u % 16):128 * (u % 16 + 1)],
                        wrs[:],
                        start=True, stop=True,
                    )
                nc.vector.tensor_tensor(
                    r_out[:, 64 * ch:64 * (ch + 1)], r_ps[:], br64[:], ALU.add
                )

            # ---- head: elementwise on [128, 32] strided SoA views ----
            # quarter H covers sample blocks n in [32H, 32H+32).
            def emit_head(H):
                def xc(d):
                    return x_sb[:, 512 * H + d:512 * (H + 1):16]

                def rc(j):
                    return r_out[:, 128 * H + j:128 * (H + 1):4]

                tiles = {}

                def st(name):
                    if name not in tiles:
                        tiles[name] = scr.tile(
                            [128, 32], F32, tag=name, name=f"{name}_{H}"
                        )
                    return tiles[name][:]

                vmul = lambda o, a, b_: nc.vector.tensor_tensor(o, a, b_, ALU.mult)
                vadd = lambda o, a, b_: nc.vector.tensor_tensor(o, a, b_, ALU.add)
                pmul = lambda o, a, b_: nc.gpsimd.tensor_tensor(o, a, b_, ALU.mult)
                padd = lambda o, a, b_: nc.gpsimd.tensor_tensor(o, a, b_, ALU.add)

                # r_tilde: rt_j = rb_{j//2} * r_j / (r_pair sum)
                vadd(st("s01"), rc(0), rc(1))
                padd(st("s23"), rc(2), rc(3))
                nc.vector.reciprocal(st("is01"), st("s01"))
                nc.vector.reciprocal(st("is23"), st("s23"))
                pmul(st("q0"), xc(12), st("is01"))
                pmul(st("q1"), xc(13), st("is23"))
                pmul(st("rt0"), st("q0"), rc(0))
                pmul(st("rt1"), st("q0"), rc(1))
                vmul(st("rt2"), st("q1"), rc(2))
                pmul(st("rt3"), st("q1"), rc(3))

                # two independent 2x2 solves
                for k, (g00, g01, g10, g11, s0, s1, rp0, rp1) in enumerate(
                    (
                        (xc(0), xc(1), xc(2), xc(3), xc(8), xc(9),
                         st("rt0"), st("rt1")),
                        (xc(4), xc(5), xc(6), xc(7), xc(10), xc(11),
                         st("rt2"), st("rt3")),
                    )
                ):
                    i00, i11 = st(f"i00_{k}"), st(f"i11_{k}")
                    nc.vector.reciprocal(i00, g00)
                    nc.vector.reciprocal(i11, g11)
                    t0, t1 = st(f"t0_{k}"), st(f"t1_{k}")
                    vmul(t0, rp0, i00)
                    pmul(t1, rp1, i11)
                    f0, f1 = st(f"f0_{k}"), st(f"f1_{k}")
                    pmul(f0, t0, g01)
                    pmul(f1, t1, g10)
                    bb0, bb1 = st(f"b0_{k}"), st(f"b1_{k}")
                    vmul(bb0, t0, s0)
                    pmul(bb1, t1, s1)
                    det = st(f"det_{k}")
                    vmul(det, f0, f1)
                    nc.vector.tensor_scalar(det, det, -1.0, 1.0, ALU.mult, ALU.add)
                    idet = st(f"idet_{k}")
                    nc.vector.reciprocal(idet, det)
                    g0t, g1t = st(f"g0_{k}"), st(f"g1_{k}")
                    pmul(g0t, f0, bb1)
                    pmul(g1t, f1, bb0)
                    n0, n1 = st(f"n0_{k}"), st(f"n1_{k}")
                    vadd(n0, bb0, g0t)
                    padd(n1, bb1, g1t)
                    pp0, pp1 = st(f"pp0_{k}"), st(f"pp1_{k}")
                    vmul(pp0, n0, idet)
                    pmul(pp1, n1, idet)
                    nc.gpsimd.tensor_scalar(
                        p_out[:, 128 * H + 2 * k:128 * (H + 1):4], pp0,
                        0.0, 0.0, ALU.max, ALU.max,
                    )
                    nc.gpsimd.tensor_scalar(
                        p_out[:, 128 * H + 2 * k + 1:128 * (H + 1):4], pp1,
                        0.0, 0.0, ALU.max, ALU.max,
                    )
                nc.sync.dma_start(
                    pov[:, 128 * H:128 * (H + 1)], p_out[:, 128 * H:128 * (H + 1)]
                )

            # ---- layer-2/3 steps (split fp16 matmuls; r interleaved) ----
            def l23_step(li, t, whi, wlo, b):
                hhi, hlo = pair[li].pop(t)
                ps = mmps.tile([NH, 512], F32, tag="mm")
                nc.tensor.matmul(ps[:], whi[:], hhi[:], start=True, stop=False)
                nc.tensor.matmul(ps[:], whi[:], hlo[:], start=False, stop=False)
                nc.tensor.matmul(ps[:], wlo[:], hhi[:], start=False, stop=True)
                if li == 0:
                    h2f = hfpool.tile([NH, 512], F32, tag="h2f", name=f"h2f{t}")
                    relu_copy(h2f[:], ps[:], b[:, 0:1])
                    make_pair(1, t, h2f[:])
                else:
                    hdst = h3[t // 4][:, 512 * (t % 4):512 * (t % 4 + 1)]
                    relu_copy(hdst, ps[:], b[:, 0:1])
                    if t % 4 == 3:
                        ch = t // 4
                        lr_pair(ch)
                        nc.sync.dma_start(
                            rov[:, 64 * ch:64 * (ch + 1)],
                            r_out[:, 64 * ch:64 * (ch + 1)],
                        )
                        if ch % 2 == 1:
                            emit_head(ch // 2)

            # ---- skewed software pipeline: L1(t) | L2(t-2) | L3(t-4) ----
            # Keeps fp16 pairs' consumers only ~2 steps behind their
            # producers so small rotating pools suffice, and overlaps all
            # three layers' PE/ACT/DVE/Pool work throughout the kernel.
            SKEW2, SKEW3 = 3, 8
            PREF = 3
            for u in range(PREF):
                l1_fetch(u)
            for t in range(NB + SKEW3):
                if t + PREF < NB:
                    l1_fetch(t + PREF)
                if t < NB:
                    l1_step(t)
                if 0 <= t - SKEW2 < NB:
                    l23_step(0, t - SKEW2, w2hi, w2lo, b2s)
                if 0 <= t - SKEW3 < NB:
                    l23_step(1, t - SKEW3, w3hi, w3lo, b3s)

    if legalize:
        _legalize_waits(nc)
    return nc


_NC_CACHE = None


def _get_nc():
    global _NC_CACHE
    if _NC_CACHE is None:
        _NC_CACHE = _build()
    return _NC_CACHE


def kernel(x, W1, b1, W2, b2, W3, b3, Wr, br, _trace=False):
    from concourse.bass_utils import run_bass_kernel_spmd

    nc = _get_nc()
    x = np.ascontiguousarray(np.asarray(x, dtype=np.float32))

    def split16(w):
        hi = w.astype(np.float16)
        lo = (w - hi.astype(np.float32)).astype(np.float16)
        return hi, lo

    W1 = np.asarray(W1, np.float32)
    W2 = np.asarray(W2, np.float32)
    W3 = np.asarray(W3, np.float32)
    w1hi, w1lo = split16(W1)
    w2hi, w2lo = split16(W2)
    w3hi, w3lo = split16(W3)
    shared = {
        "W1": np.ascontiguousarray(W1),
        "b1": np.ascontiguousarray(np.asarray(b1, np.float32)),
        "W2": np.ascontiguousarray(W2),
        "b2": np.ascontiguousarray(np.asarray(b2, np.float32)),
        "W3": np.ascontiguousarray(W3),
        "b3": np.ascontiguousarray(np.asarray(b3, np.float32)),
        "Wr": np.ascontiguousarray(np.asarray(Wr, np.float32)),
        "br": np.ascontiguousarray(np.asarray(br, np.float32)),
        "W1s_a": np.ascontiguousarray(np.concatenate([w1hi, w1lo], axis=0)),
        "W1s_b": np.ascontiguousarray(np.concatenate([w1lo, w1hi], axis=0)),
        "W2hi": np.ascontiguousarray(w2hi),
        "W2lo": np.ascontiguousarray(w2lo),
        "W3hi": np.ascontiguousarray(w3hi),
        "W3lo": np.ascontiguousarray(w3lo),
    }
    in_maps = [
        {"x": x[c * B_LOC:(c + 1) * B_LOC], **shared} for c in range(N_CORES)
    ]
    res = run_bass_kernel_spmd(
        nc, in_maps, core_ids=list(range(N_CORES)), trace=_trace
    )
    p = np.concatenate([res.results[c]["p_out"] for c in range(N_CORES)], axis=0)
    r = np.concatenate([res.results[c]["r_out"] for c in range(N_CORES)], axis=0)
    if _trace:
        kernel._last_result = res
    return p, r
